# revision 6
# baseline (speedup 1.0000x reference)
"""ComplementaryLIFNeuron on 8 Trainium2 NeuronCores (Bass, raw engine blocks).

Reference recurrence (per time step t, elementwise over [b, n, c]):
    v = v * 0.5 + x
    p = sigmoid(v / 2)          # 0.5 + 0.5*tanh(v/4)
    m = m * p
    s = (v >= 1)
    m = m + s
    q = sigmoid(m)              # 0.5 + 0.5*tanh(m/2)
    v = (v - s) - s * q
Output is s for each step, shape [(t*b), n, c].

Sharding: data-parallel over batch b=32 -> 4 rows per core; each (t, core)
block is a contiguous [4, 196*768] = [128, 4704] fp32 chunk, split into
2 column streams of 2352.

Exactness strategy (bit-identical to XLA fp32 except the tanh tables):
    m*p == 0.5 * ((tanh+1)*m)   (pow2 scaling commutes with RNE)
    track z = -((v-s) - s*q)    (negation symmetry of RNE), so
    s - v, + s*q and the next charge v' = (-0.5)*z + x all match exactly.
    t=0: v0 = x, s0 = (x>=1), m1 = s0, z0 = s0*sigma(1) + (s0 - x) with
    sigma(1) hardcoded to XLA's fp32 bit pattern. t=3: only v3, s3.

vs the previous revision:
  * RACE FIX: every load DMA has its OWN semaphore, waited at >= 16. A
    dma_start's 16 SDMA-engine increments complete independently, so on a
    shared semaphore the increments of a LATER load could satisfy an
    EARLIER load's intermediate threshold (partition-group-local stale
    data -- the observed flaky mismatches).
  * Spikes get a dedicated uint8 buffer per (t, stream): no DVE op ever
    waits on a store DMA draining (the old S-buffer reuse waits).
  * Loads issue in consumption order x0A, x1A, x0B, x1B, ... so the t=1
    charge of stream A starts ~5us earlier; smaller first slice of x0A
    so DVE starts sooner after boot.
"""

import sys
import types
import numpy as np

STEP = 4
B = 32
N = 196
C = 768
NCORES = 8
BPC = B // NCORES            # batch rows per core = 4
PELEM = BPC * N * C          # elements per (t, core) block = 602112
P = 128                      # SBUF partitions
FDFULL = PELEM // P          # 4704 free-dim columns per (t, core)
NSTREAM = 2                  # independent column streams
FD = FDFULL // NSTREAM       # 2352 columns per stream tile

SIGMA1 = float(np.uint32(0x3F3B26A8).view(np.float32))  # XLA fp32 sigmoid(1.0)

_CACHE = {}


def _ensure_axon_hooks():
    """bass_utils' trace path imports antenv.axon_hooks, absent in this image."""
    import antenv

    if "antenv.axon_hooks" not in sys.modules:
        m = types.ModuleType("antenv.axon_hooks")
        hook = [None]
        m.set_axon_ntff_profile_hook = lambda h: hook.__setitem__(0, h)
        m.get_axon_ntff_profile_hook = lambda: hook[0]
        sys.modules["antenv.axon_hooks"] = m
        antenv.axon_hooks = m
        try:
            from trn_agent_boot.trn_boot import _ntff_profile_via_ctypes

            h = _ntff_profile_via_ctypes("/opt/axon/libaxon_pjrt.so")
            if h is not None:
                m.set_axon_ntff_profile_hook(h)
        except Exception:
            pass


def build_bass():
    """Build the per-core SPMD Bass program."""
    from concourse import bass
    import concourse.mybir as mybir

    fp32 = mybir.dt.float32
    Alu = mybir.AluOpType
    Act = mybir.ActivationFunctionType

    nc = bass.Bass()
    x_ext = nc.declare_dram_parameter("x", [STEP, P, FDFULL], fp32, isOutput=False)
    s_ext = nc.declare_dram_parameter("s", [STEP, P, FDFULL], mybir.dt.uint8, isOutput=True)

    import contextlib

    ctx = contextlib.ExitStack()
    sb = {}
    for st in range(NSTREAM):
        for nm in ("X0", "X1", "z", "v", "t1", "w", "vt", "m"):
            sb[f"{nm}_{st}"] = ctx.enter_context(
                nc.sbuf_tensor(f"{nm}_{st}", [P, FD], fp32)
            )
        for t in range(STEP):
            sb[f"S{t}_{st}"] = ctx.enter_context(
                nc.sbuf_tensor(f"S{t}_{st}", [P, FD], mybir.dt.uint8)
            )

    plans = {"sync": [], "vector": [], "scalar": []}
    counts = {"vec": 0, "act": 0, "so": 0}
    mark = {}

    LOADSEMS = ["l00", "l01", "l02", "l0B", "l1A", "l1B", "l2A", "l2B", "l3A", "l3B"]
    SEMNAMES = ["vec", "act", "so"] + LOADSEMS

    def emit(engine, fn, waits=(), inc=None, label=None):
        plans[engine].append((fn, list(waits), inc))
        if inc is not None and inc[0] in counts:
            counts[inc[0]] += inc[1]
            if label is not None:
                mark[label] = (inc[0], counts[inc[0]])

    def dve(label, fn, waits=()):
        emit("vector", fn, waits=waits, inc=("vec", 1), label=label)

    def act(label, fn, waits=()):
        emit("scalar", fn, waits=waits, inc=("act", 1), label=label)

    X = lambda t, st: sb[f"X{t % 2}_{st}"]
    S = lambda t, st: sb[f"S{t}_{st}"]

    def xsrc(t, st):
        return x_ext[t][:, FD * st: FD * (st + 1)]

    def load(sem, t, st, waits=()):
        emit(
            "sync",
            lambda e, t=t, st=st: e.dma_start(out=X(t, st)[:], in_=xsrc(t, st)),
            waits=waits,
            inc=(sem, 16),
        )

    def store(t, st, waits=()):
        emit(
            "scalar",
            lambda e, t=t, st=st: e.dma_start(
                out=s_ext[t][:, FD * st: FD * (st + 1)], in_=S(t, st)[:]
            ),
            waits=waits,
            inc=("so", 16),
        )

    # ACT instruction order (marks pre-declared for DVE waits):
    # t1_1A t1_1B t2_1A t2_1B t1_2A t1_2B t2_2A t2_2B
    _actn = 0
    for _t in (1, 2):
        for _ph in ("t1", "t2"):
            for _st in range(NSTREAM):
                _actn += 1
                mark[f"{_ph}_{_t}_{_st}"] = ("act", _actn)

    # ---------------- DVE plan -------------------------------------------
    QCUTS = (0, 294, 1176, FD)
    # t = 0 stream A, progressive slices
    for qi in range(3):
        q0, q1 = QCUTS[qi], QCUTS[qi + 1]
        dve(
            f"s0A_q{qi}",
            lambda e, q0=q0, q1=q1: e.tensor_scalar(
                S(0, 0)[:, q0:q1], X(0, 0)[:, q0:q1], 1.0, None, Alu.is_ge
            ),
            waits=[(f"l0{qi}", 16)],
        )
        dve(
            f"vt0A_q{qi}",
            lambda e, q0=q0, q1=q1: e.scalar_tensor_tensor(
                sb["vt_0"][:, q0:q1], X(0, 0)[:, q0:q1], 1.0,
                X(0, 0)[:, q0:q1], Alu.is_ge, Alu.subtract,
            ),
        )
    mark["s0A"] = mark["s0A_q2"]
    mark["vt0A"] = mark["vt0A_q2"]
    dve(
        "z0A",
        lambda e: e.scalar_tensor_tensor(
            sb["z_0"][:], S(0, 0)[:], SIGMA1, sb["vt_0"][:], Alu.mult, Alu.add
        ),
    )
    dve(
        "v1A",
        lambda e: e.scalar_tensor_tensor(
            sb["v_0"][:], sb["z_0"][:], -0.5, X(1, 0)[:], Alu.mult, Alu.add
        ),
        waits=[("l1A", 16)],
    )
    # t = 0 stream B (runs under stream A's t=1 head)
    dve(
        "s0B",
        lambda e: e.tensor_scalar(S(0, 1)[:], X(0, 1)[:], 1.0, None, Alu.is_ge),
        waits=[("l0B", 16)],
    )
    dve(
        "vt0B",
        lambda e: e.scalar_tensor_tensor(
            sb["vt_1"][:], X(0, 1)[:], 1.0, X(0, 1)[:], Alu.is_ge, Alu.subtract
        ),
    )
    dve(
        "z0B",
        lambda e: e.scalar_tensor_tensor(
            sb["z_1"][:], S(0, 1)[:], SIGMA1, sb["vt_1"][:], Alu.mult, Alu.add
        ),
    )
    dve(
        "s1A",
        lambda e: e.tensor_scalar(S(1, 0)[:], sb["v_0"][:], 1.0, None, Alu.is_ge),
    )
    dve(
        "v1B",
        lambda e: e.scalar_tensor_tensor(
            sb["v_1"][:], sb["z_1"][:], -0.5, X(1, 1)[:], Alu.mult, Alu.add
        ),
        waits=[("l1B", 16)],
    )
    dve(
        "s1B",
        lambda e: e.tensor_scalar(S(1, 1)[:], sb["v_1"][:], 1.0, None, Alu.is_ge),
    )

    def midstep_tail(t):
        """w, m, vt, w2, z for both streams at step t (t = 1, 2)."""
        for st in range(NSTREAM):
            mprev = S(0, st) if t == 1 else sb[f"m_{st}"]
            dve(
                f"w{t}_{st}",
                lambda e, st=st, mprev=mprev: e.scalar_tensor_tensor(
                    sb[f"w_{st}"][:], sb[f"t1_{st}"][:], 1.0, mprev[:],
                    Alu.add, Alu.mult,
                ),
                waits=[mark[f"t1_{t}_{st}"]],
            )
        for st in range(NSTREAM):
            dve(
                f"m{t}_{st}",
                lambda e, t=t, st=st: e.scalar_tensor_tensor(
                    sb[f"m_{st}"][:], sb[f"w_{st}"][:], 0.5, S(t, st)[:],
                    Alu.mult, Alu.add,
                ),
            )
        for st in range(NSTREAM):
            dve(
                f"vt{t}_{st}",
                lambda e, st=st: e.scalar_tensor_tensor(
                    sb[f"vt_{st}"][:], sb[f"v_{st}"][:], 1.0, sb[f"v_{st}"][:],
                    Alu.is_ge, Alu.subtract,
                ),
            )
        for st in range(NSTREAM):
            dve(
                f"w2{t}_{st}",
                lambda e, t=t, st=st: e.scalar_tensor_tensor(
                    sb[f"w_{st}"][:], sb[f"t1_{st}"][:], 1.0, S(t, st)[:],
                    Alu.add, Alu.mult,
                ),
                waits=[mark[f"t2_{t}_{st}"]],
            )
        for st in range(NSTREAM):
            dve(
                f"z{t}_{st}",
                lambda e, st=st: e.scalar_tensor_tensor(
                    sb[f"z_{st}"][:], sb[f"w_{st}"][:], 0.5, sb[f"vt_{st}"][:],
                    Alu.mult, Alu.add,
                ),
            )

    midstep_tail(1)

    # t = 2 head + tail
    for st in range(NSTREAM):
        dve(
            f"v2_{st}",
            lambda e, st=st: e.scalar_tensor_tensor(
                sb[f"v_{st}"][:], sb[f"z_{st}"][:], -0.5, X(2, st)[:],
                Alu.mult, Alu.add,
            ),
            waits=[(f"l2{'AB'[st]}", 16)],
        )
    for st in range(NSTREAM):
        dve(
            f"s2_{st}",
            lambda e, st=st: e.tensor_scalar(
                S(2, st)[:], sb[f"v_{st}"][:], 1.0, None, Alu.is_ge
            ),
        )
    midstep_tail(2)

    # t = 3: halves so stores drain during the epilogue
    half = FD // 2
    HCUTS = ((0, half), (half, FD - half))
    for st in range(NSTREAM):
        for h, (h0, hsz) in enumerate(HCUTS):
            dve(
                f"v3_{st}_h{h}",
                lambda e, st=st, h0=h0, hsz=hsz: e.scalar_tensor_tensor(
                    sb[f"v_{st}"][:, h0:h0 + hsz], sb[f"z_{st}"][:, h0:h0 + hsz],
                    -0.5, X(3, st)[:, h0:h0 + hsz], Alu.mult, Alu.add,
                ),
                waits=[(f"l3{'AB'[st]}", 16)] if h == 0 else [],
            )
            dve(
                f"s3_{st}_h{h}",
                lambda e, st=st, h0=h0, hsz=hsz: e.tensor_scalar(
                    S(3, st)[:, h0:h0 + hsz], sb[f"v_{st}"][:, h0:h0 + hsz],
                    1.0, None, Alu.is_ge,
                ),
            )

    # ---------------- sync engine: loads (consumption order) --------------
    for qi in range(3):
        q0, q1 = QCUTS[qi], QCUTS[qi + 1]
        emit(
            "sync",
            lambda e, q0=q0, q1=q1: e.dma_start(
                out=X(0, 0)[:, q0:q1], in_=xsrc(0, 0)[:, q0:q1]
            ),
            inc=(f"l0{qi}", 16),
        )
    load("l1A", 1, 0)
    load("l0B", 0, 1)
    load("l1B", 1, 1)
    load("l2A", 2, 0, waits=[mark["vt0A"]])
    load("l2B", 2, 1, waits=[mark["vt0B"]])
    load("l3A", 3, 0, waits=[mark["v1A"]])
    load("l3B", 3, 1, waits=[mark["v1B"]])

    # ---------------- scalar engine: ACT + store issues -------------------
    store(0, 0, waits=[mark["s0A"]])
    act(
        "t1i_1_0",
        lambda e: e.activation(sb["t1_0"][:], sb["v_0"][:], Act.Tanh, scale=0.25),
        waits=[mark["v1A"]],
    )
    store(0, 1, waits=[mark["s0B"]])
    act(
        "t1i_1_1",
        lambda e: e.activation(sb["t1_1"][:], sb["v_1"][:], Act.Tanh, scale=0.25),
        waits=[mark["v1B"]],
    )
    store(1, 0, waits=[mark["s1A"]])
    act(
        "t2i_1_0",
        lambda e: e.activation(sb["t1_0"][:], sb["m_0"][:], Act.Tanh, scale=0.5),
        waits=[mark["m1_0"]],
    )
    act(
        "t2i_1_1",
        lambda e: e.activation(sb["t1_1"][:], sb["m_1"][:], Act.Tanh, scale=0.5),
        waits=[mark["m1_1"]],
    )
    store(1, 1, waits=[mark["s1B"]])
    act(
        "t1i_2_0",
        lambda e: e.activation(sb["t1_0"][:], sb["v_0"][:], Act.Tanh, scale=0.25),
        waits=[mark["v2_0"]],
    )
    act(
        "t1i_2_1",
        lambda e: e.activation(sb["t1_1"][:], sb["v_1"][:], Act.Tanh, scale=0.25),
        waits=[mark["v2_1"]],
    )
    store(2, 0, waits=[mark["s2_0"]])
    act(
        "t2i_2_0",
        lambda e: e.activation(sb["t1_0"][:], sb["m_0"][:], Act.Tanh, scale=0.5),
        waits=[mark["m2_0"]],
    )
    act(
        "t2i_2_1",
        lambda e: e.activation(sb["t1_1"][:], sb["m_1"][:], Act.Tanh, scale=0.5),
        waits=[mark["m2_1"]],
    )
    store(2, 1, waits=[mark["s2_1"]])
    for st in range(NSTREAM):
        for h, (h0, hsz) in enumerate(HCUTS):
            emit(
                "scalar",
                lambda e, st=st, h0=h0, hsz=hsz: e.dma_start(
                    out=s_ext[3][:, FD * st + h0: FD * st + h0 + hsz],
                    in_=S(3, st)[:, h0:h0 + hsz],
                ),
                waits=[mark[f"s3_{st}_h{h}"]],
                inc=("so", 16),
            )

    final_so = counts["so"]

    # ---------------------------------------------------------------------
    import contextlib as _cl
    with _cl.ExitStack() as semctx:
        block = semctx.enter_context(nc.Block())
        sems = {nm: semctx.enter_context(nc.semaphore(nm)) for nm in SEMNAMES}

        def run_plan(engine_handle, plan, final_wait=None):
            for fn, waits, inc in plan:
                for (s, v) in waits:
                    engine_handle.wait_ge(sems[s], v)
                ins = fn(engine_handle)
                if inc is not None:
                    ins.then_inc(sems[inc[0]], inc[1])
            if final_wait is not None:
                engine_handle.wait_ge(sems[final_wait[0]], final_wait[1])

        @block.sync
        def _(e):
            run_plan(e, plans["sync"])

        @block.tensor
        def _(e):
            pass

        @block.gpsimd
        def _(e):
            pass

        @block.vector
        def _(e):
            run_plan(e, plans["vector"])

        @block.scalar
        def _(e):
            run_plan(e, plans["scalar"], final_wait=("so", final_so))

    ctx.close()
    return nc


def _get_program():
    if "nc" not in _CACHE:
        _ensure_axon_hooks()
        _CACHE["nc"] = build_bass()
    return _CACHE["nc"]


def shard_inputs(x_seq):
    """x_seq [(t*b), n, c] -> per-core [STEP, P, FDFULL] contiguous blocks."""
    xt = np.ascontiguousarray(x_seq).reshape(STEP, B, N * C)
    maps = []
    for k in range(NCORES):
        blk = xt[:, k * BPC: (k + 1) * BPC, :].reshape(STEP, P, FDFULL)
        maps.append({"x": np.ascontiguousarray(blk)})
    return maps


def unshard_outputs(results):
    """Per-core [STEP, P, FDFULL] spike blocks -> [(t*b), n, c]."""
    out = np.empty((STEP, B, N * C), dtype=np.float32)
    for k in range(NCORES):
        blk = results[k]["s"].reshape(STEP, BPC, N * C)
        out[:, k * BPC: (k + 1) * BPC, :] = blk
    return out.reshape(STEP * B, N, C)


def kernel(x_seq, step, _trace=False):
    assert int(step) == STEP
    assert x_seq.shape == (STEP * B, N, C)
    x_seq = np.asarray(x_seq, dtype=np.float32)

    from concourse.bass_utils import run_bass_kernel_spmd

    nc = _get_program()
    in_maps = shard_inputs(x_seq)
    res = run_bass_kernel_spmd(nc, in_maps, list(range(NCORES)), trace=_trace)
    out = unshard_outputs(res.results)
    if _trace:
        return out, res
    return out


# revision 8
# speedup vs baseline: 1.0062x; 1.0062x over previous
"""ComplementaryLIFNeuron on 8 Trainium2 NeuronCores (Bass, raw engine blocks).

Reference recurrence (per time step t, elementwise over [b, n, c]):
    v = v * 0.5 + x
    p = sigmoid(v / 2)          # 0.5 + 0.5*tanh(v/4)
    m = m * p
    s = (v >= 1)
    m = m + s
    q = sigmoid(m)              # 0.5 + 0.5*tanh(m/2)
    v = (v - s) - s * q
Output is s for each step, shape [(t*b), n, c].

Sharding: data-parallel over batch b=32 -> 4 rows per core; each (t, core)
block is a contiguous [4, 196*768] = [128, 4704] fp32 chunk, split into
2 column streams of 2352.

Exactness strategy (bit-identical to XLA fp32 except the tanh tables):
    m*p == 0.5 * ((tanh+1)*m)   (pow2 scaling commutes with RNE)
    track z = -((v-s) - s*q)    (negation symmetry of RNE), so
    s - v, + s*q and the next charge v' = (-0.5)*z + x all match exactly.
    t=0: v0 = x, s0 = (x>=1), m1 = s0, z0 = s0*sigma(1) + (s0 - x) with
    sigma(1) hardcoded to XLA's fp32 bit pattern. t=3: only v3, s3.

vs the previous revision:
  * RACE FIX: every load DMA has its OWN semaphore, waited at >= 16. A
    dma_start's 16 SDMA-engine increments complete independently, so on a
    shared semaphore the increments of a LATER load could satisfy an
    EARLIER load's intermediate threshold (partition-group-local stale
    data -- the observed flaky mismatches).
  * Spikes get a dedicated uint8 buffer per (t, stream): no DVE op ever
    waits on a store DMA draining (the old S-buffer reuse waits).
  * Loads issue in consumption order x0A, x1A, x0B, x1B, ... so the t=1
    charge of stream A starts ~5us earlier; smaller first slice of x0A
    so DVE starts sooner after boot.
"""

import sys
import types
import numpy as np

STEP = 4
B = 32
N = 196
C = 768
NCORES = 8
BPC = B // NCORES            # batch rows per core = 4
PELEM = BPC * N * C          # elements per (t, core) block = 602112
P = 128                      # SBUF partitions
FDFULL = PELEM // P          # 4704 free-dim columns per (t, core)
NSTREAM = 2                  # independent column streams
FD = FDFULL // NSTREAM       # 2352 columns per stream tile

SIGMA1 = float(np.uint32(0x3F3B26A8).view(np.float32))  # XLA fp32 sigmoid(1.0)

_CACHE = {}


def _ensure_axon_hooks():
    """bass_utils' trace path imports antenv.axon_hooks, absent in this image."""
    import antenv

    if "antenv.axon_hooks" not in sys.modules:
        m = types.ModuleType("antenv.axon_hooks")
        hook = [None]
        m.set_axon_ntff_profile_hook = lambda h: hook.__setitem__(0, h)
        m.get_axon_ntff_profile_hook = lambda: hook[0]
        sys.modules["antenv.axon_hooks"] = m
        antenv.axon_hooks = m
        try:
            from trn_agent_boot.trn_boot import _ntff_profile_via_ctypes

            h = _ntff_profile_via_ctypes("/opt/axon/libaxon_pjrt.so")
            if h is not None:
                m.set_axon_ntff_profile_hook(h)
        except Exception:
            pass


def build_bass():
    """Build the per-core SPMD Bass program."""
    from concourse import bass
    import concourse.mybir as mybir

    fp32 = mybir.dt.float32
    Alu = mybir.AluOpType
    Act = mybir.ActivationFunctionType

    nc = bass.Bass()
    x_ext = nc.declare_dram_parameter("x", [STEP, P, FDFULL], fp32, isOutput=False)
    s_ext = nc.declare_dram_parameter("s", [STEP, P, FDFULL], mybir.dt.uint8, isOutput=True)

    import contextlib

    ctx = contextlib.ExitStack()
    sb = {}
    for st in range(NSTREAM):
        for nm in ("X0", "X1", "z", "v", "t1", "w", "vt", "m"):
            sb[f"{nm}_{st}"] = ctx.enter_context(
                nc.sbuf_tensor(f"{nm}_{st}", [P, FD], fp32)
            )
        for t in range(STEP):
            sb[f"S{t}_{st}"] = ctx.enter_context(
                nc.sbuf_tensor(f"S{t}_{st}", [P, FD], mybir.dt.uint8)
            )

    plans = {"sync": [], "vector": [], "scalar": []}
    counts = {"vec": 0, "act": 0, "so": 0}
    mark = {}

    LOADSEMS = ["l00", "l01", "l02", "l0B", "l1A", "l1B", "l2A", "l2B", "l3A", "l3B"]
    SEMNAMES = ["vec", "act", "so"] + LOADSEMS

    def emit(engine, fn, waits=(), inc=None, label=None):
        plans[engine].append((fn, list(waits), inc))
        if inc is not None and inc[0] in counts:
            counts[inc[0]] += inc[1]
            if label is not None:
                mark[label] = (inc[0], counts[inc[0]])

    def dve(label, fn, waits=()):
        emit("vector", fn, waits=waits, inc=("vec", 1), label=label)

    def act(label, fn, waits=()):
        emit("scalar", fn, waits=waits, inc=("act", 1), label=label)

    X = lambda t, st: sb[f"X{t % 2}_{st}"]
    S = lambda t, st: sb[f"S{t}_{st}"]

    def xsrc(t, st):
        return x_ext[t][:, FD * st: FD * (st + 1)]

    def load(sem, t, st, waits=()):
        emit(
            "sync",
            lambda e, t=t, st=st: e.dma_start(out=X(t, st)[:], in_=xsrc(t, st)),
            waits=waits,
            inc=(sem, 16),
        )

    def store(t, st, waits=()):
        emit(
            "scalar",
            lambda e, t=t, st=st: e.dma_start(
                out=s_ext[t][:, FD * st: FD * (st + 1)], in_=S(t, st)[:]
            ),
            waits=waits,
            inc=("so", 16),
        )

    # ACT instruction order (marks pre-declared for DVE waits):
    # t1_1A t1_1B t2_1A t2_1B t1_2A t1_2B t2_2A t2_2B
    _actn = 0
    for _t in (1, 2):
        for _ph in ("t1", "t2"):
            for _st in range(NSTREAM):
                _actn += 1
                mark[f"{_ph}_{_t}_{_st}"] = ("act", _actn)

    # ---------------- DVE plan -------------------------------------------
    QCUTS = (0, 294, 1176, FD)
    # t = 0 stream A, progressive slices
    for qi in range(3):
        q0, q1 = QCUTS[qi], QCUTS[qi + 1]
        dve(
            f"s0A_q{qi}",
            lambda e, q0=q0, q1=q1: e.tensor_scalar(
                S(0, 0)[:, q0:q1], X(0, 0)[:, q0:q1], 1.0, None, Alu.is_ge
            ),
            waits=[(f"l0{qi}", 16)],
        )
        dve(
            f"vt0A_q{qi}",
            lambda e, q0=q0, q1=q1: e.scalar_tensor_tensor(
                sb["vt_0"][:, q0:q1], X(0, 0)[:, q0:q1], 1.0,
                X(0, 0)[:, q0:q1], Alu.is_ge, Alu.subtract,
            ),
        )
    mark["s0A"] = mark["s0A_q2"]
    mark["vt0A"] = mark["vt0A_q2"]
    dve(
        "z0A",
        lambda e: e.scalar_tensor_tensor(
            sb["z_0"][:], S(0, 0)[:], SIGMA1, sb["vt_0"][:], Alu.mult, Alu.add
        ),
    )
    dve(
        "v1A",
        lambda e: e.scalar_tensor_tensor(
            sb["v_0"][:], sb["z_0"][:], -0.5, X(1, 0)[:], Alu.mult, Alu.add
        ),
        waits=[("l1A", 16)],
    )
    # t = 0 stream B (runs under stream A's t=1 head)
    dve(
        "s0B",
        lambda e: e.tensor_scalar(S(0, 1)[:], X(0, 1)[:], 1.0, None, Alu.is_ge),
        waits=[("l0B", 16)],
    )
    dve(
        "vt0B",
        lambda e: e.scalar_tensor_tensor(
            sb["vt_1"][:], X(0, 1)[:], 1.0, X(0, 1)[:], Alu.is_ge, Alu.subtract
        ),
    )
    dve(
        "z0B",
        lambda e: e.scalar_tensor_tensor(
            sb["z_1"][:], S(0, 1)[:], SIGMA1, sb["vt_1"][:], Alu.mult, Alu.add
        ),
    )
    dve(
        "s1A",
        lambda e: e.tensor_scalar(S(1, 0)[:], sb["v_0"][:], 1.0, None, Alu.is_ge),
    )
    dve(
        "v1B",
        lambda e: e.scalar_tensor_tensor(
            sb["v_1"][:], sb["z_1"][:], -0.5, X(1, 1)[:], Alu.mult, Alu.add
        ),
        waits=[("l1B", 16)],
    )
    dve(
        "s1B",
        lambda e: e.tensor_scalar(S(1, 1)[:], sb["v_1"][:], 1.0, None, Alu.is_ge),
    )

    def midstep_tail(t):
        """w, m, vt, w2, z for both streams at step t (t = 1, 2)."""
        for st in range(NSTREAM):
            mprev = S(0, st) if t == 1 else sb[f"m_{st}"]
            dve(
                f"w{t}_{st}",
                lambda e, st=st, mprev=mprev: e.scalar_tensor_tensor(
                    sb[f"w_{st}"][:], sb[f"t1_{st}"][:], 1.0, mprev[:],
                    Alu.add, Alu.mult,
                ),
                waits=[mark[f"t1_{t}_{st}"]],
            )
        for st in range(NSTREAM):
            dve(
                f"m{t}_{st}",
                lambda e, t=t, st=st: e.scalar_tensor_tensor(
                    sb[f"m_{st}"][:], sb[f"w_{st}"][:], 0.5, S(t, st)[:],
                    Alu.mult, Alu.add,
                ),
            )
        for st in range(NSTREAM):
            dve(
                f"vt{t}_{st}",
                lambda e, st=st: e.scalar_tensor_tensor(
                    sb[f"vt_{st}"][:], sb[f"v_{st}"][:], 1.0, sb[f"v_{st}"][:],
                    Alu.is_ge, Alu.subtract,
                ),
            )
        for st in range(NSTREAM):
            dve(
                f"w2{t}_{st}",
                lambda e, t=t, st=st: e.scalar_tensor_tensor(
                    sb[f"w_{st}"][:], sb[f"t1_{st}"][:], 1.0, S(t, st)[:],
                    Alu.add, Alu.mult,
                ),
                waits=[mark[f"t2_{t}_{st}"]],
            )
        for st in range(NSTREAM):
            dve(
                f"z{t}_{st}",
                lambda e, st=st: e.scalar_tensor_tensor(
                    sb[f"z_{st}"][:], sb[f"w_{st}"][:], 0.5, sb[f"vt_{st}"][:],
                    Alu.mult, Alu.add,
                ),
            )

    midstep_tail(1)

    # t = 2 head + tail
    for st in range(NSTREAM):
        dve(
            f"v2_{st}",
            lambda e, st=st: e.scalar_tensor_tensor(
                sb[f"v_{st}"][:], sb[f"z_{st}"][:], -0.5, X(2, st)[:],
                Alu.mult, Alu.add,
            ),
            waits=[(f"l2{'AB'[st]}", 16)],
        )
    for st in range(NSTREAM):
        dve(
            f"s2_{st}",
            lambda e, st=st: e.tensor_scalar(
                S(2, st)[:], sb[f"v_{st}"][:], 1.0, None, Alu.is_ge
            ),
        )
    midstep_tail(2)

    # t = 3: sliced so stores drain during the epilogue; the last (stream B)
    # tile uses quarters so the final store is as small as possible.
    half = FD // 2
    quar = FD // 4
    CUTS = {
        0: ((0, half), (half, FD - half)),
        1: ((0, half), (half, quar), (half + quar, FD - half - quar)),
    }
    for st in range(NSTREAM):
        for h, (h0, hsz) in enumerate(CUTS[st]):
            dve(
                f"v3_{st}_h{h}",
                lambda e, st=st, h0=h0, hsz=hsz: e.scalar_tensor_tensor(
                    sb[f"v_{st}"][:, h0:h0 + hsz], sb[f"z_{st}"][:, h0:h0 + hsz],
                    -0.5, X(3, st)[:, h0:h0 + hsz], Alu.mult, Alu.add,
                ),
                waits=[(f"l3{'AB'[st]}", 16)] if h == 0 else [],
            )
            dve(
                f"s3_{st}_h{h}",
                lambda e, st=st, h0=h0, hsz=hsz: e.tensor_scalar(
                    S(3, st)[:, h0:h0 + hsz], sb[f"v_{st}"][:, h0:h0 + hsz],
                    1.0, None, Alu.is_ge,
                ),
            )

    # ---------------- sync engine: loads (consumption order) --------------
    for qi in range(3):
        q0, q1 = QCUTS[qi], QCUTS[qi + 1]
        emit(
            "sync",
            lambda e, q0=q0, q1=q1: e.dma_start(
                out=X(0, 0)[:, q0:q1], in_=xsrc(0, 0)[:, q0:q1]
            ),
            inc=(f"l0{qi}", 16),
        )
    load("l1A", 1, 0)
    load("l0B", 0, 1)
    load("l1B", 1, 1)
    load("l2A", 2, 0, waits=[mark["vt0A"]])
    load("l2B", 2, 1, waits=[mark["vt0B"]])
    load("l3A", 3, 0, waits=[mark["v1A"]])
    load("l3B", 3, 1, waits=[mark["v1B"]])

    # ---------------- scalar engine: ACT + store issues -------------------
    store(0, 0, waits=[mark["s0A"]])
    act(
        "t1i_1_0",
        lambda e: e.activation(sb["t1_0"][:], sb["v_0"][:], Act.Tanh, scale=0.25),
        waits=[mark["v1A"]],
    )
    store(0, 1, waits=[mark["s0B"]])
    act(
        "t1i_1_1",
        lambda e: e.activation(sb["t1_1"][:], sb["v_1"][:], Act.Tanh, scale=0.25),
        waits=[mark["v1B"]],
    )
    store(1, 0, waits=[mark["s1A"]])
    act(
        "t2i_1_0",
        lambda e: e.activation(sb["t1_0"][:], sb["m_0"][:], Act.Tanh, scale=0.5),
        waits=[mark["m1_0"]],
    )
    act(
        "t2i_1_1",
        lambda e: e.activation(sb["t1_1"][:], sb["m_1"][:], Act.Tanh, scale=0.5),
        waits=[mark["m1_1"]],
    )
    store(1, 1, waits=[mark["s1B"]])
    act(
        "t1i_2_0",
        lambda e: e.activation(sb["t1_0"][:], sb["v_0"][:], Act.Tanh, scale=0.25),
        waits=[mark["v2_0"]],
    )
    act(
        "t1i_2_1",
        lambda e: e.activation(sb["t1_1"][:], sb["v_1"][:], Act.Tanh, scale=0.25),
        waits=[mark["v2_1"]],
    )
    store(2, 0, waits=[mark["s2_0"]])
    act(
        "t2i_2_0",
        lambda e: e.activation(sb["t1_0"][:], sb["m_0"][:], Act.Tanh, scale=0.5),
        waits=[mark["m2_0"]],
    )
    act(
        "t2i_2_1",
        lambda e: e.activation(sb["t1_1"][:], sb["m_1"][:], Act.Tanh, scale=0.5),
        waits=[mark["m2_1"]],
    )
    store(2, 1, waits=[mark["s2_1"]])
    # final spike stores alternate between the two HWDGE queues (scalar +
    # sync) so the drain is ~2x faster; "so" is only ever final-waited at
    # its cumulative total, which is race-safe on a shared semaphore.
    _q = 0
    for st in range(NSTREAM):
        for h, (h0, hsz) in enumerate(CUTS[st]):
            engine = ("scalar", "sync")[_q % 2]
            _q += 1
            emit(
                engine,
                lambda e, st=st, h0=h0, hsz=hsz: e.dma_start(
                    out=s_ext[3][:, FD * st + h0: FD * st + h0 + hsz],
                    in_=S(3, st)[:, h0:h0 + hsz],
                ),
                waits=[mark[f"s3_{st}_h{h}"]],
                inc=("so", 16),
            )

    final_so = counts["so"]

    # ---------------------------------------------------------------------
    import contextlib as _cl
    with _cl.ExitStack() as semctx:
        block = semctx.enter_context(nc.Block())
        sems = {nm: semctx.enter_context(nc.semaphore(nm)) for nm in SEMNAMES}

        def run_plan(engine_handle, plan, final_wait=None):
            for fn, waits, inc in plan:
                for (s, v) in waits:
                    engine_handle.wait_ge(sems[s], v)
                ins = fn(engine_handle)
                if inc is not None:
                    ins.then_inc(sems[inc[0]], inc[1])
            if final_wait is not None:
                engine_handle.wait_ge(sems[final_wait[0]], final_wait[1])

        @block.sync
        def _(e):
            run_plan(e, plans["sync"])

        @block.tensor
        def _(e):
            pass

        @block.gpsimd
        def _(e):
            pass

        @block.vector
        def _(e):
            run_plan(e, plans["vector"])

        @block.scalar
        def _(e):
            run_plan(e, plans["scalar"], final_wait=("so", final_so))

    ctx.close()
    return nc


def _get_program():
    if "nc" not in _CACHE:
        _ensure_axon_hooks()
        _CACHE["nc"] = build_bass()
    return _CACHE["nc"]


def shard_inputs(x_seq):
    """x_seq [(t*b), n, c] -> per-core [STEP, P, FDFULL] contiguous blocks."""
    xt = np.ascontiguousarray(x_seq).reshape(STEP, B, N * C)
    maps = []
    for k in range(NCORES):
        blk = xt[:, k * BPC: (k + 1) * BPC, :].reshape(STEP, P, FDFULL)
        maps.append({"x": np.ascontiguousarray(blk)})
    return maps


def unshard_outputs(results):
    """Per-core [STEP, P, FDFULL] spike blocks -> [(t*b), n, c]."""
    out = np.empty((STEP, B, N * C), dtype=np.float32)
    for k in range(NCORES):
        blk = results[k]["s"].reshape(STEP, BPC, N * C)
        out[:, k * BPC: (k + 1) * BPC, :] = blk
    return out.reshape(STEP * B, N, C)


def kernel(x_seq, step, _trace=False):
    assert int(step) == STEP
    assert x_seq.shape == (STEP * B, N, C)
    x_seq = np.asarray(x_seq, dtype=np.float32)

    from concourse.bass_utils import run_bass_kernel_spmd

    nc = _get_program()
    in_maps = shard_inputs(x_seq)
    res = run_bass_kernel_spmd(nc, in_maps, list(range(NCORES)), trace=_trace)
    out = unshard_outputs(res.results)
    if _trace:
        return out, res
    return out
